# revision 1
# baseline (speedup 1.0000x reference)
"""Trainium2 Bass kernel for nn_DestSelectionPolicy (GNN edge softmax).

Math: att[e,c] = relu(x[row_e]@W[c,:64] + x[col_e]@W[c,64:] + b[c]);
segment-softmax over edges grouped by row (destination), per channel;
mask amount==0 edges; sum the 2 channels -> out[e].

Identity used: exp(s)/sum(exp(s)) == exp(s-m)/sum(exp(s-m)); s=relu(.) is in
[0, ~5] so the unshifted exp is fp32-safe and matches the reference within
rounding (the reference's +1e-16 denominator eps is relative 1e-16 here since
the max term contributes exp(0)=1 to the sum).

Sharding: edges partitioned by destination row range (6250 rows/core x 8
cores) so each node's softmax segment is device-local; x is replicated.
Per core the device:
  1. builds a node-pair table uv[pair r] = entries for nodes 2r and 2r+1,
     each entry [u0+b0, u1+b1, v0, v1] (u = x@W[:, :64].T, v = x@W[:, 64:].T)
     via PE matmuls, written 256B-strided to DRAM,
  2. per [128-node x Dt-slot] grid tile, one batched SWDGE dma_gather
     (InstDMAGatherAnt, mlp Q7 library) fetches the 32B pair row for every
     edge slot (idx = col//2, int16), the pair half is parity-selected on
     DVE, then relu/exp on ACT, masked segment-sum + divide on DVE, and the
     per-edge grid is written back.
Host packs edges into the grids (nodes sorted by degree so per-tile slot
counts Dt hug the real degrees) and scatters grid outputs back to edge
order."""
import sys

sys.path.insert(0, "/opt/trn_rl_repo")

import numpy as np
import concourse.bass as bass
import concourse.bacc as bacc
import concourse.mybir as mybir
from concourse import ap_utils
from concourse._compat import round_up_to_multiple, exact_div
from concourse.bass_utils import run_bass_kernel_spmd
from concourse.tile import TileContext
from concourse.vector_clock import ScopedClock
import concourse.tile as tile_mod

N = 50000
E = 1600000
D = 64
NC = 8
RPC = N // NC
RP = 6272
NT = RP // 128
NROWS_TBL = 50176
XT = NROWS_TBL // 128
NPAIR = NROWS_TBL // 2
F32 = mybir.dt.float32
I32 = mybir.dt.int32
I16 = mybir.dt.int16

_MAXW = 1


def _patched_drain_and_barrier(self, tick_clock, wait_clock):
    carrier = self.nc.sync.nop(nofuse=True, hint="drain_waits")
    wait_clock.add_sem_waits(
        carrier.ins, ScopedClock({None: tick_clock.global_clock})
    )
    si = carrier.ins.sync_info
    waits = list(si.on_wait) if si is not None else []
    if si is not None:
        si.on_wait = waits[:_MAXW]
    for i in range(_MAXW, len(waits), _MAXW):
        nop = self.nc.sync.nop(nofuse=True, hint="drain_waits")
        if nop.ins.sync_info is None:
            nop.ins.sync_info = mybir.SyncInfo(on_wait=[], on_update=[])
        nop.ins.sync_info.on_wait = waits[i : i + _MAXW]
    self.nc.sync.drain()
    self.nc.all_engine_barrier()
    assert self.sems is not None
    popped = self.nc._tile_sem_poison_stack.pop()
    assert popped is self._sem_poison
    self.nc.clear_and_free_semaphores(list(self.sems.allocated().values()))
    self.nc.all_engine_barrier()


tile_mod.TileContext._drain_and_barrier = _patched_drain_and_barrier


def _split_waits(nc, maxw: int = _MAXW):
    for fn in nc.m.functions:
        for bb in fn.blocks:
            new_insts = []
            for inst in bb.instructions:
                si = inst.sync_info
                if si is not None and si.on_wait and len(si.on_wait) > maxw:
                    waits = list(si.on_wait)
                    si.on_wait = waits[-maxw:]
                    for i in range(0, len(waits) - maxw, maxw):
                        new_insts.append(
                            mybir.InstNoOp(
                                name=nc.get_next_instruction_name(),
                                engine=inst.engine,
                                sync_info=mybir.SyncInfo(
                                    on_wait=waits[i : i + maxw], on_update=[]
                                ),
                                text_hint="wait_split",
                            )
                        )
                new_insts.append(inst)
            bb.instructions[:] = new_insts


def _dma_gather(eng, out_ap, in_ap, idxs_ap, num_idxs, elem_size, elem_step):
    """InstDMAGatherAnt without bass's %256 elem-size assert (that restriction
    is for transpose mode; the ucode handles small elems — HW-verified)."""
    assert idxs_ap.dtype == I16
    assert ap_utils.ap_is_contiguous(out_ap.ap[1:])
    assert ap_utils.ap_is_contiguous(idxs_ap.ap[1:])
    assert in_ap.ap[-1][1] == out_ap.ap[-1][1] == elem_size
    assert out_ap.ap[0][1] * out_ap.ap[1][1] == round_up_to_multiple(num_idxs, 128)
    assert in_ap.ap[0][0] == elem_step
    stride_bytes_256 = exact_div(elem_step * mybir.dt.size(in_ap.dtype), 256)
    _in_ap = eng.lower_ap_dma(in_ap, for_custom_bir_dma=True)
    _idxs_ap = eng.lower_ap(idxs_ap)
    _out_ap = eng.lower_ap(out_ap)
    return eng.add_instruction(
        mybir.InstDMAGatherAnt(
            name=eng.bass.get_next_instruction_name(),
            ins=[*_in_ap, _idxs_ap, eng.lower_val_access(eng.to_reg(num_idxs))],
            outs=[_out_ap],
            transpose=False,
            num_idxs=num_idxs,
            elem_size=elem_size,
            stride_bytes_256=stride_bytes_256,
            gen_mode=0,
            single_packet=False,
            queue_num=0,
            sbuf_tokens_per_rank=0,
            sbuf_free_dim_per_rank=0,
            sbuf_free_dim_pad_per_rank=0,
            sbuf_byte_offset=0,
        )
    )


_CACHE = {}


def _build_nc(dts):
    W_slots = max(dts)
    offs = np.concatenate([[0], np.cumsum([8 * d for d in dts])]).astype(int)
    totw = int(offs[-1])
    nc = bacc.Bacc("TRN2")
    x_t = nc.declare_dram_parameter("x_t", [D, NROWS_TBL], F32, isOutput=False)
    wcat = nc.declare_dram_parameter("wcat", [D, 4], F32, isOutput=False)
    btile = nc.declare_dram_parameter("btile", [128, 64], F32, isOutput=False)
    idx16 = nc.declare_dram_parameter("idx16", [128, totw], I16, isOutput=False)
    u_idx16 = nc.declare_dram_parameter("u_idx16", [128, RP // 16], I16, isOutput=False)
    u_par4 = nc.declare_dram_parameter("u_par4", [128, NT * 4], F32, isOutput=False)
    pvm_g = nc.declare_dram_parameter("pvm_g", [RP, 2, W_slots], F32, isOutput=False)
    padc = nc.declare_dram_parameter("padc", [128, NT], F32, isOutput=False)
    out_g = nc.declare_dram_parameter("out_g", [RP, W_slots], F32, isOutput=True)
    uv = nc.dram_tensor("uv_tbl", [NPAIR, 64], F32)

    G = 16
    with TileContext(nc) as tc:
        with (
            tc.tile_pool(name="consts", bufs=1) as cpool,
            tc.tile_pool(name="xc", bufs=3) as xpool,
            tc.tile_pool(name="ps", bufs=4, space="PSUM") as pspool,
            tc.tile_pool(name="st", bufs=3) as stpool,
            tc.tile_pool(name="edge", bufs=3) as epool,
            tc.tile_pool(name="vals", bufs=3) as vpool,
            tc.tile_pool(name="small", bufs=4) as spool,
        ):
            wc = cpool.tile([D, 4], F32, tag="wc")
            nc.sync.dma_start(out=wc[:], in_=wcat[:])
            bt = cpool.tile([128, 64], F32, tag="bt")
            nc.sync.dma_start(out=bt[:], in_=btile[:])

            # phase 1: pair table. x_t columns are host-permuted so that in
            # each 128-node block, partitions 0:64 hold even nodes (pair col
            # 0:4) and 64:128 hold odd nodes (pair col 4:8).
            for g0 in range(0, XT, G):
                gn = min(G, XT - g0)
                xc = xpool.tile([D, 128 * gn], F32, tag="xc")
                nc.sync.dma_start(
                    out=xc[:], in_=x_t[:, g0 * 128 : (g0 + gn) * 128]
                )
                st = stpool.tile([128, 4 * gn], F32, tag="st")
                ps = pspool.tile([128, 4 * gn], F32, tag="ps")
                for g in range(gn):
                    nc.tensor.matmul(
                        out=ps[:, g * 4 : (g + 1) * 4],
                        lhsT=xc[:, g * 128 : (g + 1) * 128],
                        rhs=wc[:],
                        start=True,
                        stop=True,
                    )
                nc.vector.tensor_add(
                    out=st[:], in0=ps[:], in1=bt[:, 0 : 4 * gn]
                )
                pbase = g0 * 64
                nc.sync.dma_start(
                    out=uv[pbase : pbase + gn * 64, 0:4].rearrange(
                        "(g q) c -> q g c", q=64
                    ),
                    in_=st[0:64, :].rearrange("p (g c) -> p g c", c=4),
                )
                nc.sync.dma_start(
                    out=uv[pbase : pbase + gn * 64, 4:8].rearrange(
                        "(g q) c -> q g c", q=64
                    ),
                    in_=st[64:128, :].rearrange("p (g c) -> p g c", c=4),
                )

            padt = cpool.tile([1, 8], F32, tag="padt")
            nc.vector.memset(padt[:], -1.0e30)
            nc.sync.dma_start(out=uv[NPAIR - 1 : NPAIR, 0:8], in_=padt[:])

            # phase 2: one batched gather for all grid rows' u entries
            uixt = cpool.tile([128, RP // 16], I16, tag="uixt")
            nc.sync.dma_start(out=uixt[:], in_=u_idx16[:])
            upt = cpool.tile([128, NT * 4], F32, tag="upt")
            nc.sync.dma_start(out=upt[:], in_=u_par4[:])
            pct = cpool.tile([128, NT], F32, tag="pct")
            nc.sync.dma_start(out=pct[:], in_=padc[:])
            ur_all = cpool.tile([128, NT * 8], F32, tag="ur_all")
            _dma_gather(
                nc.gpsimd,
                out_ap=ur_all[:].rearrange("p (t c) -> p t c", c=8),
                in_ap=uv[:, 0:8],
                idxs_ap=uixt[:],
                num_idxs=RP,
                elem_size=8,
                elem_step=64,
            )
            ur3 = ur_all[:].rearrange("p (t c) -> p t c", c=8)
            ut_all = cpool.tile([128, NT * 4], F32, tag="ut_all")
            ut3 = ut_all[:].rearrange("p (t c) -> p t c", c=4)
            up3 = upt[:].rearrange("p (t c) -> p t c", c=4)
            nc.vector.tensor_sub(out=ut3, in0=ur3[:, :, 4:8], in1=ur3[:, :, 0:4])
            nc.vector.tensor_mul(out=ut3, in0=ut3, in1=up3)
            nc.vector.tensor_add(out=ut3, in0=ut3, in1=ur3[:, :, 0:4])

            for t in range(NT):
                dt = dts[t]
                r0 = t * 128
                ixt = epool.tile([128, 8 * dt], I16, tag="ixt")
                nc.sync.dma_start(
                    out=ixt[:], in_=idx16[:, offs[t] : offs[t + 1]]
                )
                vals = vpool.tile([128, dt * 8], F32, tag="vals")
                _dma_gather(
                    nc.gpsimd,
                    out_ap=vals[:].rearrange("p (d c) -> p d c", c=8),
                    in_ap=uv[:, 0:8],
                    idxs_ap=ixt[:],
                    num_idxs=128 * dt,
                    elem_size=8,
                    elem_step=64,
                )
                pvm = epool.tile([128, 2 * dt], F32, tag="pvm")
                nc.sync.dma_start(
                    out=pvm[:].rearrange("p (k d) -> p k d", k=2),
                    in_=pvm_g[r0 : r0 + 128, :, 0:dt],
                )
                pt = pvm[:, 0:dt]
                mt = pvm[:, dt : 2 * dt]

                v3 = vals[:].rearrange("p (d c) -> p d c", c=8)
                o = epool.tile([128, dt], F32, tag="o")
                den = spool.tile([128, 2], F32, tag="den")
                rec = spool.tile([128, 2], F32, tag="rec")
                for c in range(2):
                    sc = epool.tile([128, dt], F32, tag=f"s{c}")
                    nc.vector.tensor_sub(
                        out=sc[:], in0=v3[:, :, 6 + c], in1=v3[:, :, 2 + c]
                    )
                    nc.vector.tensor_mul(out=sc[:], in0=sc[:], in1=pt)
                    nc.vector.tensor_add(out=sc[:], in0=sc[:], in1=v3[:, :, 2 + c])
                    ec = epool.tile([128, dt], F32, tag=f"e{c}")
                    nc.scalar.activation(
                        out=ec[:],
                        in_=sc[:],
                        func=mybir.ActivationFunctionType.Relu,
                        bias=ut_all[:, t * 4 + c : t * 4 + c + 1],
                    )
                    nc.scalar.activation(
                        out=ec[:], in_=ec[:], func=mybir.ActivationFunctionType.Exp
                    )
                    nc.vector.tensor_reduce(
                        out=den[:, c : c + 1],
                        in_=ec[:],
                        axis=mybir.AxisListType.X,
                        op=mybir.AluOpType.add,
                    )
                    nc.vector.tensor_scalar_sub(
                        out=den[:, c : c + 1],
                        in0=den[:, c : c + 1],
                        scalar1=pct[:, t : t + 1],
                    )
                    nc.vector.reciprocal(
                        out=rec[:, c : c + 1], in_=den[:, c : c + 1]
                    )
                    if c == 0:
                        nc.vector.tensor_scalar_mul(
                            out=o[:], in0=ec[:], scalar1=rec[:, 0:1]
                        )
                    else:
                        ec2 = epool.tile([128, dt], F32, tag="ec2")
                        nc.vector.tensor_scalar_mul(
                            out=ec2[:], in0=ec[:], scalar1=rec[:, 1:2]
                        )
                        nc.vector.tensor_add(out=o[:], in0=o[:], in1=ec2[:])
                nc.vector.tensor_mul(out=o[:], in0=o[:], in1=mt)
                nc.sync.dma_start(out=out_g[r0 : r0 + 128, 0:dt], in_=o[:])

    _split_waits(nc)
    nc.finalize()
    return nc, offs, W_slots


def _wrap16(flat):
    # index j consumed from (j%16, j//16), replicated across the 8 Q7 cores
    n = flat.size
    w = flat.reshape(n // 16, 16).T.astype(np.int16)
    return np.tile(w, (8, 1))


def kernel(x, edge_index, actual_amount, W, b):
    x = np.asarray(x, np.float32)
    edge_index = np.asarray(edge_index)
    amt = np.asarray(actual_amount).ravel()
    W = np.asarray(W, np.float32)
    b = np.asarray(b, np.float32)
    row = edge_index[0].astype(np.int64)
    col = edge_index[1].astype(np.int64)

    # x transposed, padded, and pair-permuted: block-local partitions
    # [0:64]=even nodes, [64:128]=odd nodes
    x_pad = np.zeros((D, NROWS_TBL), np.float32)
    x_pad[:, :N] = x.T
    blk = np.arange(NROWS_TBL).reshape(XT, 128)
    perm_cols = np.concatenate(
        [blk[:, 0::2], blk[:, 1::2]], axis=1
    ).ravel()  # position (t, q<64) <- node t*128+2q ; (t, 64+q) <- +2q+1
    x_t = x_pad[:, perm_cols]
    wcat = np.stack([W[0, :D], W[1, :D], W[0, D:], W[1, D:]], axis=1).astype(
        np.float32
    )
    btile = np.tile(
        np.array([b[0], b[1], 0.0, 0.0], np.float32)[None, :], (128, 16)
    )

    per_core = []
    dts_all = np.zeros((NC, NT), np.int64)
    for c in range(NC):
        sel = np.nonzero((row >= c * RPC) & (row < (c + 1) * RPC))[0]
        r_loc = row[sel] - c * RPC
        deg = np.bincount(r_loc, minlength=RPC)
        perm = np.argsort(-deg, kind="stable")
        inv = np.empty(RPC, np.int64)
        inv[perm] = np.arange(RPC)
        prow = inv[r_loc]
        order = np.argsort(prow, kind="stable")
        sel_o = sel[order]
        prow_o = prow[order]
        counts = np.bincount(prow_o, minlength=RPC)
        coffs = np.concatenate([[0], np.cumsum(counts)[:-1]])
        slot = np.arange(len(sel_o)) - coffs[prow_o]
        deg_sorted = deg[perm]
        for t in range(NT):
            lo = t * 128
            dts_all[c, t] = deg_sorted[lo] if lo < RPC else 0
        per_core.append((sel_o, prow_o, slot, perm, deg_sorted))

    dts = tuple(int(max(1, d)) for d in dts_all.max(axis=0))

    if dts not in _CACHE:
        _CACHE[dts] = _build_nc(dts)
    nc, offs, W_slots = _CACHE[dts]
    totw = int(offs[-1])

    in_maps = []
    for c in range(NC):
        sel_o, prow_o, slot, perm, deg_sorted_arr = per_core[c]
        colg = np.full((RP, W_slots), 2 * (NPAIR - 1), np.int64)
        colg[prow_o, slot] = col[sel_o]
        pvm_g = np.zeros((RP, 2, W_slots), np.float32)
        pvm_g[prow_o, 0, slot] = (col[sel_o] % 2).astype(np.float32)
        pvm_g[prow_o, 1, slot] = (amt[sel_o] != 0).astype(np.float32)
        idx16 = np.zeros((128, totw), np.int16)
        for t in range(NT):
            dt = int((offs[t + 1] - offs[t]) // 8)
            # j = i*128 + p  ->  pair id of grid (p, i)
            flat = (colg[t * 128 : (t + 1) * 128, 0:dt] // 2).T.ravel()
            idx16[:, offs[t] : offs[t + 1]] = _wrap16(flat.astype(np.int16))
        gids = np.zeros(RP, np.int64)
        gids[:RPC] = c * RPC + perm
        u_idx16 = _wrap16((gids // 2).astype(np.int16))
        u_par4 = np.repeat(
            (gids % 2).astype(np.float32).reshape(NT, 128).T, 4, axis=1
        ).copy()
        nslots = np.zeros(RP, np.float32)
        nslots[:RPC] = deg_sorted_arr
        dtrow = np.repeat(np.array(dts, np.float32), 128)
        padc_all = (dtrow - nslots).reshape(NT, 128).T.copy()
        in_maps.append(
            {
                "x_t": x_t,
                "wcat": wcat,
                "btile": btile,
                "idx16": idx16,
                "u_idx16": u_idx16,
                "u_par4": u_par4,
                "pvm_g": pvm_g,
                "padc": padc_all,
            }
        )

    import time as _time

    _t0 = _time.time()
    res = run_bass_kernel_spmd(nc, in_maps, list(range(NC)))
    global LAST_RUN_WALL
    LAST_RUN_WALL = _time.time() - _t0

    out = np.zeros(E, np.float32)
    for c in range(NC):
        sel_o, prow_o, slot, _, _ = per_core[c]
        grid = np.asarray(res.results[c]["out_g"])
        out[sel_o] = grid[prow_o, slot]
    return out



# revision 2
# speedup vs baseline: 1.0128x; 1.0128x over previous
"""Trainium2 Bass kernel for nn_DestSelectionPolicy (GNN edge softmax), v2.

Math: att[e,c] = relu(u[row_e,c] + v[col_e,c]) with u = x@Wl.T + b, v = x@Wr.T;
segment-softmax over edges grouped by row (destination), per channel; mask
amount==0 edges (applied host-side at scatter); sum the 2 channels -> out[e].

v1 -> v2 changes (the call is axon-transfer-bound at ~80MB/s up / ~40MB/s
down, so the redesign is a data diet):
  * x is no longer replicated (was 12.8MB x 8 cores): each core uploads a
    1.6MB slice, computes its 1/8 of the per-node [u0+b0,u1+b1,v0,v1] pair
    table on PE, and an HBM AllGather replicates the 256B-strided table.
  * edge gather indices upload un-replicated ([16, n] wrap instead of the
    [128, n] x8-tiled layout the Q7 gather ucode wants; broadcast on-device
    with 8 DMA copies) and carry the col parity in the int16 sign bit
    (stripped with bitwise_and, extracted with logical_shift_right on DVE).
  * the amount==0 mask moved to the host-side scatter (outputs for masked
    edges are simply dropped), killing the per-slot parity/mask f32 planes.
  * the output grid is packed [128, sum(dts)] and written with one DMA.
  * run_bass_via_pjrt is patched with a jit-cache so repeat calls skip the
    client-side retrace/recompile (compile_bir_kernel + XLA) that cost
    ~0.3s+ per call; device work is unchanged.
Remaining per-call traffic: ~2.2MB up + 0.85MB down per core.
"""
import sys

sys.path.insert(0, "/opt/trn_rl_repo")

import numpy as np
import concourse.bass as bass
import concourse.bacc as bacc
import concourse.mybir as mybir
from concourse import ap_utils
from concourse._compat import round_up_to_multiple, exact_div
from concourse.bass_utils import run_bass_kernel_spmd
from concourse.tile import TileContext
from concourse.vector_clock import ScopedClock
import concourse.tile as tile_mod
import concourse.bass2jax as bass2jax

N = 50000
E = 1600000
D = 64
NC = 8
RPC = N // NC          # 6250 edge-partition rows per core
RP = 6272              # padded to 49 x 128
NT = RP // 128         # 49 row tiles
TBL_N = NC * RP        # 50176 node-table entries (incl. zero pad)
NPAIR = TBL_N // 2     # 25088
PPC = NPAIR // NC      # 3136 pairs contributed per core
DEAD = NPAIR - 1       # dead pair (-1e30 entries) for padding slots
F32 = mybir.dt.float32
F16 = mybir.dt.float16
I16 = mybir.dt.int16

_MAXW = 1


def _patched_drain_and_barrier(self, tick_clock, wait_clock):
    carrier = self.nc.sync.nop(nofuse=True, hint="drain_waits")
    wait_clock.add_sem_waits(
        carrier.ins, ScopedClock({None: tick_clock.global_clock})
    )
    si = carrier.ins.sync_info
    waits = list(si.on_wait) if si is not None else []
    if si is not None:
        si.on_wait = waits[:_MAXW]
    for i in range(_MAXW, len(waits), _MAXW):
        nop = self.nc.sync.nop(nofuse=True, hint="drain_waits")
        if nop.ins.sync_info is None:
            nop.ins.sync_info = mybir.SyncInfo(on_wait=[], on_update=[])
        nop.ins.sync_info.on_wait = waits[i : i + _MAXW]
    self.nc.sync.drain()
    self.nc.all_engine_barrier()
    assert self.sems is not None
    popped = self.nc._tile_sem_poison_stack.pop()
    assert popped is self._sem_poison
    self.nc.clear_and_free_semaphores(list(self.sems.allocated().values()))
    self.nc.all_engine_barrier()


tile_mod.TileContext._drain_and_barrier = _patched_drain_and_barrier


def _split_waits(nc, maxw: int = _MAXW):
    for fn in nc.m.functions:
        for bb in fn.blocks:
            new_insts = []
            for inst in bb.instructions:
                si = inst.sync_info
                if si is not None and si.on_wait and len(si.on_wait) > maxw:
                    waits = list(si.on_wait)
                    si.on_wait = waits[-maxw:]
                    for i in range(0, len(waits) - maxw, maxw):
                        new_insts.append(
                            mybir.InstNoOp(
                                name=nc.get_next_instruction_name(),
                                engine=inst.engine,
                                sync_info=mybir.SyncInfo(
                                    on_wait=waits[i : i + maxw], on_update=[]
                                ),
                                text_hint="wait_split",
                            )
                        )
                new_insts.append(inst)
            bb.instructions[:] = new_insts


def _dma_gather(eng, out_ap, in_ap, idxs_ap, num_idxs, elem_size, elem_step):
    """InstDMAGatherAnt without bass's %256 elem-size assert (that restriction
    is for transpose mode; the ucode handles small elems — HW-verified)."""
    assert idxs_ap.dtype == I16
    assert ap_utils.ap_is_contiguous(out_ap.ap[1:])
    assert ap_utils.ap_is_contiguous(idxs_ap.ap[1:])
    assert in_ap.ap[-1][1] == out_ap.ap[-1][1] == elem_size
    assert out_ap.ap[0][1] * out_ap.ap[1][1] == round_up_to_multiple(num_idxs, 128)
    assert in_ap.ap[0][0] == elem_step
    stride_bytes_256 = exact_div(elem_step * mybir.dt.size(in_ap.dtype), 256)
    _in_ap = eng.lower_ap_dma(in_ap, for_custom_bir_dma=True)
    _idxs_ap = eng.lower_ap(idxs_ap)
    _out_ap = eng.lower_ap(out_ap)
    return eng.add_instruction(
        mybir.InstDMAGatherAnt(
            name=eng.bass.get_next_instruction_name(),
            ins=[*_in_ap, _idxs_ap, eng.lower_val_access(eng.to_reg(num_idxs))],
            outs=[_out_ap],
            transpose=False,
            num_idxs=num_idxs,
            elem_size=elem_size,
            stride_bytes_256=stride_bytes_256,
            gen_mode=0,
            single_packet=False,
            queue_num=0,
            sbuf_tokens_per_rank=0,
            sbuf_free_dim_per_rank=0,
            sbuf_free_dim_pad_per_rank=0,
            sbuf_byte_offset=0,
        )
    )


# ---------------------------------------------------------------------------
# jit-cache for run_bass_via_pjrt: the stock version builds a fresh closure
# and jax.jit per call, so every call re-runs neuronx_cc_hook (client-side
# compile_bir_kernel + dve table gen) and XLA compilation. Cache the jitted
# executable keyed on the Bass object; per-call work is then just concat +
# transfer + execute + download.
_PJRT_FN_CACHE = {}


def _run_bass_via_pjrt_cached(nc, in_maps, n_cores):
    import jax
    from jax.sharding import Mesh, PartitionSpec
    from jax.experimental.shard_map import shard_map

    key = (id(nc), n_cores)
    ent = _PJRT_FN_CACHE.get(key)
    if ent is None:
        bass2jax.install_neuronx_cc_hook()
        assert nc.dbg_addr is None, "debug kernels not supported by the cache"
        partition_name = (
            nc.partition_id_tensor.name if nc.partition_id_tensor else None
        )
        in_names, out_names, out_avals = [], [], []
        for alloc in nc.m.functions[0].allocations:
            if not isinstance(alloc, mybir.MemoryLocationSet):
                continue
            assert alloc.memorylocations
            name = alloc.memorylocations[0].name
            if alloc.kind == "ExternalInput":
                if name != partition_name:
                    in_names.append(name)
            elif alloc.kind == "ExternalOutput":
                out_names.append(name)
                out_avals.append(
                    jax.core.ShapedArray(
                        tuple(alloc.tensor_shape), mybir.dt.np(alloc.dtype)
                    )
                )
        n_params = len(in_names)
        n_outs = len(out_avals)
        all_in_names = list(in_names) + list(out_names)
        if partition_name is not None:
            all_in_names.append(partition_name)
        donate = tuple(range(n_params, n_params + n_outs))

        def _body(*args):
            operands = list(args)
            if partition_name is not None:
                operands.append(bass2jax.partition_id_tensor())
            outs = bass2jax._bass_exec_p.bind(
                *operands,
                out_avals=tuple(out_avals),
                in_names=tuple(all_in_names),
                out_names=tuple(out_names),
                lowering_input_output_aliases=(),
                sim_require_finite=True,
                sim_require_nnan=True,
                nc=nc,
            )
            return tuple(outs)

        devices = jax.devices()[:n_cores]
        assert len(devices) == n_cores
        mesh = Mesh(np.asarray(devices), ("core",))
        in_specs = (PartitionSpec("core"),) * (n_params + n_outs)
        out_specs = (PartitionSpec("core"),) * n_outs
        sharded = jax.jit(
            shard_map(
                _body,
                mesh=mesh,
                in_specs=in_specs,
                out_specs=out_specs,
                check_rep=False,
            ),
            keep_unused=True,
        )
        # output scratch buffers: uploaded once and reused (NOT donated);
        # this kernel writes every element of its outputs, so stale
        # contents can't leak — saves re-uploading zeros each call
        from jax.sharding import NamedSharding

        zeros_dev = [
            jax.device_put(
                np.zeros((n_cores * a.shape[0], *a.shape[1:]), a.dtype),
                NamedSharding(mesh, PartitionSpec("core")),
            )
            for a in out_avals
        ]
        ent = (sharded, in_names, out_names, out_avals, n_params, zeros_dev)
        _PJRT_FN_CACHE[key] = ent

    sharded, in_names, out_names, out_avals, n_params, zeros_dev = ent
    concat_in = [
        np.concatenate([np.asarray(m[name]) for m in in_maps], axis=0)
        for name in in_names
    ]
    out_arrs = sharded(*concat_in, *zeros_dev)
    # materialize each output ONCE: np.asarray on a sharded jax array
    # re-fetches the shards on every call (observed 8x the download time)
    out_np = [
        np.asarray(a).reshape(n_cores, *out_avals[i].shape)
        for i, a in enumerate(out_arrs)
    ]
    return [
        {name: out_np[i][c] for i, name in enumerate(out_names)}
        for c in range(n_cores)
    ]


bass2jax.run_bass_via_pjrt = _run_bass_via_pjrt_cached


_CACHE = {}


def _build_nc(dts):
    offd = np.concatenate([[0], np.cumsum(dts)]).astype(int)
    S = int(offd[-1])          # packed grid columns
    S16 = 8 * S                # idx wrap columns
    nc = bacc.Bacc("TRN2", num_devices=NC)
    # all small per-core inputs travel in one i16 blob (each extra input
    # array costs ~5ms of fixed per-array transfer overhead over axon);
    # section offsets in i16 units, f32 sections 4B-aligned
    o_uidx = 16 * S16
    o_padc = o_uidx + 16 * 8 * NT
    o_bt = o_padc + 2 * 128 * NT
    o_wc = o_bt + 2 * 64 * 64
    TOT16 = o_wc + 64 * 4
    xs = nc.declare_dram_parameter("xs", [D, RP], F16, isOutput=False)
    blob = nc.declare_dram_parameter("blob", [1, TOT16], I16, isOutput=False)
    out_pk = nc.declare_dram_parameter("out_pk", [128, S], F16, isOutput=True)
    tblm = nc.dram_tensor("tblm", [PPC, 64], F32)
    tbl = nc.dram_tensor("tbl", [NPAIR, 64], F32)
    idxp16 = blob[0, 0 : 16 * S16].rearrange("(p c) -> p c", c=S16)
    uidxp16 = blob[0, o_uidx : o_uidx + 16 * 8 * NT].rearrange(
        "(p c) -> p c", c=8 * NT
    )
    padc = blob[0, o_padc : o_padc + 2 * 128 * NT].bitcast(F32).rearrange(
        "(p c) -> p c", c=NT
    )
    btile = blob[0, o_bt : o_bt + 2 * 64 * 64].bitcast(F32).rearrange(
        "(p c) -> p c", c=64
    )
    wcat = blob[0, o_wc : o_wc + 64 * 4].bitcast(F16).rearrange(
        "(p c) -> p c", c=4
    )

    G = 7  # phase-1 blocks per matmul group (NT = 49 = 7*7)
    with TileContext(nc) as tc:
        with (
            tc.tile_pool(name="consts", bufs=1) as cpool,
            tc.tile_pool(name="ps", bufs=4, space="PSUM") as pspool,
            tc.tile_pool(name="st", bufs=3) as stpool,
            tc.tile_pool(name="edge", bufs=3) as epool,
            tc.tile_pool(name="vals", bufs=3) as vpool,
            tc.tile_pool(name="small", bufs=4) as spool,
        ):
            wc = cpool.tile([D, 4], F16, tag="wc")
            nc.sync.dma_start(out=wc[:], in_=wcat)
            bt = cpool.tile([64, 64], F32, tag="bt")
            nc.sync.dma_start(out=bt[:], in_=btile)

            # phase 1: this core's 1/8 of the pair table. xs columns are
            # host-permuted so block t has even nodes in cols [128t,128t+64)
            # and odd in [128t+64, 128t+128); two matmuls per block write
            # [u0+b0,u1+b1,v0,v1] for the even/odd node into one partition,
            # giving 32B-contiguous pair entries.
            xst = cpool.tile([D, RP], F16, tag="xst")
            nc.sync.dma_start(out=xst[:], in_=xs[:])
            for g0 in range(0, NT, G):
                ps = pspool.tile([64, 8 * G], F32, tag="ps")
                for g in range(G):
                    t = g0 + g
                    nc.tensor.matmul(
                        out=ps[:, 8 * g : 8 * g + 4],
                        lhsT=xst[:, 128 * t : 128 * t + 64],
                        rhs=wc[:],
                        start=True,
                        stop=True,
                    )
                    nc.tensor.matmul(
                        out=ps[:, 8 * g + 4 : 8 * g + 8],
                        lhsT=xst[:, 128 * t + 64 : 128 * t + 128],
                        rhs=wc[:],
                        start=True,
                        stop=True,
                    )
                stg = stpool.tile([64, 8 * G], F32, tag="stg")
                nc.vector.tensor_add(
                    out=stg[:], in0=ps[:], in1=bt[:, 0 : 8 * G]
                )
                # reorder each 8-col group [u_e(2) v_e(2) u_o(2) v_o(2)]
                # -> [v_e v_o u_e u_o] so the edge gather can fetch just
                # the leading 16B of each entry (elem_size=4)
                st2 = stpool.tile([64, 8 * G], F32, tag="st2")
                stv = stg[:].rearrange("q (g h x) -> q g h x", h=2, x=4)
                st4 = st2[:].rearrange("q (g h2 c) -> q g h2 c", h2=4, c=2)
                nc.vector.tensor_scalar(
                    out=st4[:, :, 0:2, :], in0=stv[:, :, :, 2:4],
                    scalar1=0.0, scalar2=None, op0=mybir.AluOpType.add,
                )
                nc.vector.tensor_scalar(
                    out=st4[:, :, 2:4, :], in0=stv[:, :, :, 0:2],
                    scalar1=0.0, scalar2=None, op0=mybir.AluOpType.add,
                )
                nc.sync.dma_start(
                    out=tblm[g0 * 64 : (g0 + G) * 64, 0:8].rearrange(
                        "(g q) c -> q g c", q=64
                    ),
                    in_=st2[:].rearrange("q (g c) -> q g c", c=8),
                )

            # replicate the table across cores (core c contributed pairs
            # [c*PPC, (c+1)*PPC))
            nc.gpsimd.collective_compute(
                "AllGather",
                mybir.AluOpType.bypass,
                replica_groups=[list(range(NC))],
                ins=[tblm[:, :]],
                outs=[tbl[:, :]],
            )
            # dead pair: padding slots gather this entry; relu clamps the
            # -1e30 to 0 so each pad slot contributes exactly 1.0 to the
            # denominator, corrected via padc.
            padt = cpool.tile([1, 8], F32, tag="padt")
            nc.vector.memset(padt[:], -1.0e30)
            nc.sync.dma_start(out=tbl[DEAD : DEAD + 1, 0:8], in_=padt[:])

            # edge/u indices: upload the [16, n] wrap once, broadcast x8 on
            # device (the Q7 gather ucode reads a [128, n] x8-replicated
            # layout), then strip the parity sign bit. A second broadcast
            # shifts group h's copy left by h columns: wrap slot of grid
            # (p, d) is 8d + p//16, so the shifted copy turns the parity
            # wrap->grid shuffle into one stride-8 AP over all 128
            # partitions (engines can't address 16-partition groups at
            # offset 16, but DMA can).
            idxpb = cpool.tile([128, S16], I16, tag="idxpb")
            parpb = cpool.tile([128, S16], I16, tag="parpb")
            uxpb = cpool.tile([128, 8 * NT], I16, tag="uxpb")
            upapb = cpool.tile([128, 8 * NT], I16, tag="upapb")
            for h in range(8):
                nc.sync.dma_start(
                    out=idxpb[16 * h : 16 * h + 16, :], in_=idxp16
                )
                nc.sync.dma_start(
                    out=parpb[16 * h : 16 * h + 16, 0 : S16 - h],
                    in_=idxp16[:, h:S16],
                )
                nc.sync.dma_start(
                    out=uxpb[16 * h : 16 * h + 16, :], in_=uidxp16
                )
                nc.sync.dma_start(
                    out=upapb[16 * h : 16 * h + 16, 0 : 8 * NT - h],
                    in_=uidxp16[:, h : 8 * NT],
                )
            # NB: the shift is arithmetic on HW despite the name -> 0 / -1;
            # the i16->f32 converts below use mult -1.0 to get 0.0 / 1.0
            parb = cpool.tile([128, S16], I16, tag="parb")
            nc.vector.tensor_scalar(
                out=parb[:], in0=parpb[:], scalar1=15, scalar2=None,
                op0=mybir.AluOpType.logical_shift_right,
            )
            uparb = cpool.tile([128, 8 * NT], I16, tag="uparb")
            nc.vector.tensor_scalar(
                out=uparb[:], in0=upapb[:], scalar1=15, scalar2=None,
                op0=mybir.AluOpType.logical_shift_right,
            )
            # the gather ucode mishandles idx APs at a non-zero column
            # offset (HW-probed), so every gather gets a fresh idx tile
            # starting at its base; the strip writes into it anyway.
            uxb = cpool.tile([128, 8 * NT], I16, tag="uxb")
            nc.vector.tensor_scalar(
                out=uxb[:], in0=uxpb[:], scalar1=0x7FFF, scalar2=None,
                op0=mybir.AluOpType.bitwise_and,
            )


            pct = cpool.tile([128, NT], F32, tag="pct")
            nc.sync.dma_start(out=pct[:], in_=padc)

            # row-node u entries: one gather for all RP grid rows
            ur_all = cpool.tile([128, NT * 8], F32, tag="ur_all")
            _dma_gather(
                nc.gpsimd,
                out_ap=ur_all[:].rearrange("p (t c) -> p t c", c=8),
                in_ap=tbl[:, 0:8],
                idxs_ap=uxb[:],
                num_idxs=RP,
                elem_size=8,
                elem_step=64,
            )
            # u parity: shifted wrap -> grid layout [128, NT] in one op
            upg = cpool.tile([128, NT], F32, tag="upg")
            nc.vector.tensor_scalar(
                out=upg[:],
                in0=uparb[:].rearrange("p (t e) -> p t e", e=8)[:, :, 0],
                scalar1=-1.0, scalar2=None, op0=mybir.AluOpType.mult,
            )
            # parity-select the row node's u0+b0 / u1+b1 -> ut_all[:, 2t+c]
            ur3 = ur_all[:].rearrange("p (t c) -> p t c", c=8)
            ut_all = cpool.tile([128, NT * 2], F32, tag="ut_all")
            ut3 = ut_all[:].rearrange("p (t c) -> p t c", c=2)
            for c in range(2):
                nc.vector.tensor_sub(
                    out=ut3[:, :, c], in0=ur3[:, :, 6 + c], in1=ur3[:, :, 4 + c]
                )
                nc.vector.tensor_mul(out=ut3[:, :, c], in0=ut3[:, :, c], in1=upg[:])
                nc.vector.tensor_add(
                    out=ut3[:, :, c], in0=ut3[:, :, c], in1=ur3[:, :, 4 + c]
                )

            o_all = cpool.tile([128, S], F16, tag="o_all")
            parb3 = parb[:].rearrange("p (s e) -> p s e", e=8)
            for t in range(NT):
                dt = int(dts[t])
                o16 = 8 * int(offd[t])
                it = vpool.tile([128, 8 * dt], I16, tag="it")
                nc.vector.tensor_scalar(
                    out=it[:], in0=idxpb[:, o16 : o16 + 8 * dt],
                    scalar1=0x7FFF, scalar2=None,
                    op0=mybir.AluOpType.bitwise_and,
                )
                vals = vpool.tile([128, dt * 4], F32, tag="vals")
                _dma_gather(
                    nc.gpsimd,
                    out_ap=vals[:].rearrange("p (d c) -> p d c", c=4),
                    in_ap=tbl[:, 0:4],
                    idxs_ap=it[:],
                    num_idxs=128 * dt,
                    elem_size=4,
                    elem_step=64,
                )
                # col parity: shifted wrap -> grid [128, dt] f32, one op
                parg = epool.tile([128, dt], F32, tag="parg")
                nc.vector.tensor_scalar(
                    out=parg[:],
                    in0=parb3[:, int(offd[t]) : int(offd[t]) + dt, 0],
                    scalar1=-1.0, scalar2=None, op0=mybir.AluOpType.mult,
                )
                v3 = vals[:].rearrange("p (d c) -> p d c", c=4)
                of = epool.tile([128, dt], F32, tag="of")
                o = of[:]
                den = spool.tile([128, 2], F32, tag="den")
                rec = spool.tile([128, 2], F32, tag="rec")
                for c in range(2):
                    sc = epool.tile([128, dt], F32, tag=f"s{c}")
                    nc.vector.tensor_sub(
                        out=sc[:], in0=v3[:, :, 2 + c], in1=v3[:, :, c]
                    )
                    nc.vector.tensor_mul(out=sc[:], in0=sc[:], in1=parg[:])
                    nc.vector.tensor_add(out=sc[:], in0=sc[:], in1=v3[:, :, c])
                    ec = epool.tile([128, dt], F32, tag=f"e{c}")
                    nc.scalar.activation(
                        out=ec[:],
                        in_=sc[:],
                        func=mybir.ActivationFunctionType.Relu,
                        bias=ut_all[:, 2 * t + c : 2 * t + c + 1],
                    )
                    nc.scalar.activation(
                        out=ec[:], in_=ec[:], func=mybir.ActivationFunctionType.Exp
                    )
                    nc.vector.tensor_reduce(
                        out=den[:, c : c + 1],
                        in_=ec[:],
                        axis=mybir.AxisListType.X,
                        op=mybir.AluOpType.add,
                    )
                    nc.vector.tensor_scalar_sub(
                        out=den[:, c : c + 1],
                        in0=den[:, c : c + 1],
                        scalar1=pct[:, t : t + 1],
                    )
                    nc.vector.reciprocal(
                        out=rec[:, c : c + 1], in_=den[:, c : c + 1]
                    )
                    if c == 0:
                        nc.vector.tensor_scalar_mul(
                            out=o, in0=ec[:], scalar1=rec[:, 0:1]
                        )
                    else:
                        ec2 = epool.tile([128, dt], F32, tag="ec2")
                        nc.vector.tensor_scalar_mul(
                            out=ec2[:], in0=ec[:], scalar1=rec[:, 1:2]
                        )
                        nc.vector.tensor_add(out=o, in0=o, in1=ec2[:])
                nc.vector.tensor_scalar(
                    out=o_all[:, int(offd[t]) : int(offd[t]) + dt],
                    in0=o, scalar1=0.0, scalar2=None,
                    op0=mybir.AluOpType.add,
                )
            nc.sync.dma_start(out=out_pk[:, :], in_=o_all[:])

    _split_waits(nc)
    nc.finalize()
    return nc, offd


def _wrap16(flat):
    # gather index j is consumed from (j%16, j//16) of the wrap
    n = flat.size
    return np.ascontiguousarray(flat.reshape(n // 16, 16).T)


def kernel(x, edge_index, actual_amount, W, b):
    x = np.asarray(x, np.float32)
    edge_index = np.asarray(edge_index)
    amt = np.asarray(actual_amount).ravel()
    W = np.asarray(W, np.float32)
    b = np.asarray(b, np.float32)
    row = edge_index[0].astype(np.int64)
    col = edge_index[1].astype(np.int64)

    # x transposed, padded, block-pair permuted (per 128-node block: even
    # nodes -> partitions 0:64, odd -> 64:128), then sliced per core: core c
    # computes table nodes [c*RP, (c+1)*RP)
    x_pad = np.zeros((D, TBL_N), np.float32)
    x_pad[:, :N] = x.T
    blk = np.arange(TBL_N).reshape(TBL_N // 128, 128)
    perm_cols = np.concatenate([blk[:, 0::2], blk[:, 1::2]], axis=1).ravel()
    x_t = x_pad[:, perm_cols].astype(np.float16)
    wcat = np.stack([W[0, :D], W[1, :D], W[0, D:], W[1, D:]], axis=1).astype(
        np.float16
    )
    btile = np.tile(
        np.array([b[0], b[1], 0.0, 0.0, b[0], b[1], 0.0, 0.0], np.float32)[None, :],
        (64, 8),
    )

    per_core = []
    dts_all = np.zeros((NC, NT), np.int64)
    for c in range(NC):
        sel = np.nonzero((row >= c * RPC) & (row < (c + 1) * RPC))[0]
        r_loc = row[sel] - c * RPC
        deg = np.bincount(r_loc, minlength=RPC)
        perm = np.argsort(-deg, kind="stable")
        inv = np.empty(RPC, np.int64)
        inv[perm] = np.arange(RPC)
        prow = inv[r_loc]
        order = np.argsort(prow, kind="stable")
        sel_o = sel[order]
        prow_o = prow[order]
        counts = np.bincount(prow_o, minlength=RPC)
        coffs = np.concatenate([[0], np.cumsum(counts)[:-1]])
        slot = np.arange(len(sel_o)) - coffs[prow_o]
        deg_sorted = deg[perm]
        for t in range(NT):
            lo = t * 128
            dts_all[c, t] = deg_sorted[lo] if lo < RPC else 0
        per_core.append((sel_o, prow_o, slot, perm, deg_sorted))

    dts = tuple(int(max(1, d)) for d in dts_all.max(axis=0))

    if dts not in _CACHE:
        _CACHE[dts] = _build_nc(dts)
    nc, offd = _CACHE[dts]
    S = int(offd[-1])

    in_maps = []
    for c in range(NC):
        sel_o, prow_o, slot, perm, deg_sorted_arr = per_core[c]
        col_sel = col[sel_o]
        # packed edge index: pair id | parity<<15, dead pair in pad slots
        pk = np.full((RP, int(max(dts))), DEAD, np.uint16)
        pk[prow_o, slot] = (
            (col_sel >> 1) | ((col_sel & 1) << 15)
        ).astype(np.uint16)
        idxp16 = np.zeros((16, 8 * S), np.int16)
        for t in range(NT):
            dt = dts[t]
            flat = pk[t * 128 : (t + 1) * 128, 0:dt].T.ravel()
            idxp16[:, 8 * int(offd[t]) : 8 * int(offd[t]) + 8 * dt] = _wrap16(
                flat
            ).view(np.int16)
        gids = np.zeros(RP, np.int64)
        gids[:RPC] = c * RPC + perm
        upk = ((gids >> 1) | ((gids & 1) << 15)).astype(np.uint16)
        uidxp16 = _wrap16(upk).view(np.int16)
        nslots = np.zeros(RP, np.float32)
        nslots[:RPC] = deg_sorted_arr
        dtrow = np.repeat(np.array(dts, np.float32), 128)
        padc_all = (dtrow - nslots).reshape(NT, 128).T.copy()
        blob = np.concatenate(
            [
                idxp16.ravel(),
                uidxp16.ravel(),
                np.ascontiguousarray(padc_all).view(np.int16).ravel(),
                np.ascontiguousarray(btile).view(np.int16).ravel(),
                np.ascontiguousarray(wcat).view(np.int16).ravel(),
            ]
        )[None, :]
        in_maps.append(
            {
                "xs": np.ascontiguousarray(x_t[:, c * RP : (c + 1) * RP]),
                "blob": blob,
            }
        )

    import time as _time

    _t0 = _time.time()
    res = run_bass_kernel_spmd(nc, in_maps, list(range(NC)))
    global LAST_RUN_WALL
    LAST_RUN_WALL = _time.time() - _t0

    offd_np = np.asarray(offd, np.int64)
    out = np.zeros(E, np.float32)
    for c in range(NC):
        sel_o, prow_o, slot, _, _ = per_core[c]
        grid = np.asarray(res.results[c]["out_pk"]).astype(np.float32)
        vals = grid[prow_o % 128, offd_np[prow_o // 128] + slot]
        vals[amt[sel_o] == 0] = 0.0
        out[sel_o] = vals
    return out
